# revision 2
# baseline (speedup 1.0000x reference)
import numpy as np
import jax
import jax.numpy as jnp

# nn_AdjustableLengthAttention — criss-cross attention with adjustable length mask.
# Full shapes (hardcoded per spec): x1,x2,x3 [B=8, C=512, H=64, W=64] fp32;
# Wq,Wk [64,512]; bq,bk [64]; Wv [512,512]; bv [512]; gamma scalar; length int.
# Sharding: data-parallel over batch B across the 8 NeuronCores (one batch
# element per core via jax.pmap); small conv weights replicated.

_B, _C, _H, _W = 8, 512, 64, 64
_CQ = _C // 8

_pmapped_cache = {}


def _single(x1, x2, x3, Wq, bq, Wk, bk, Wv, bv, gamma, mH, mW):
    # One batch element: x1,x2,x3 [C,H,W]
    h = _H
    q = jnp.einsum("chw,oc->ohw", x1, Wq) + bq[:, None, None]  # [CQ,H,W]
    k = jnp.einsum("chw,oc->ohw", x2, Wk) + bk[:, None, None]
    v = jnp.einsum("chw,oc->ohw", x3, Wv) + bv[:, None, None]  # [C,H,W]

    qH = jnp.transpose(q, (2, 1, 0)) * mH  # [W,H,CQ]
    kH = jnp.transpose(k, (2, 0, 1)) * mH  # [W,CQ,H]
    eH = jnp.einsum("whc,wcg->whg", qH, kH)  # [W,H,H]
    eH = jnp.where(jnp.eye(h, dtype=bool), -jnp.inf, eH)
    eH = jnp.transpose(eH, (1, 0, 2))  # [H,W,H]

    qW = jnp.transpose(q, (1, 2, 0)) * mW  # [H,W,CQ]
    kW = jnp.transpose(k, (1, 0, 2)) * mW  # [H,CQ,W]
    eW = jnp.einsum("hwc,hcg->hwg", qW, kW)  # [H,W,W]

    att = jax.nn.softmax(jnp.concatenate([eH, eW], axis=2), axis=2)  # [H,W,H+W]
    attH = jnp.transpose(att[..., :h], (1, 0, 2))  # [W,H,H]
    attW = att[..., h:]  # [H,W,W]

    vH = jnp.transpose(v, (2, 0, 1))  # [W,C,H]
    vW = jnp.transpose(v, (1, 0, 2))  # [H,C,W]
    outH = jnp.einsum("wcj,wij->wci", vH, attH)  # [W,C,H]
    outH = jnp.transpose(outH, (1, 2, 0))  # [C,H,W]
    outW = jnp.einsum("hcj,hij->hci", vW, attW)  # [H,C,W]
    outW = jnp.transpose(outW, (1, 0, 2))  # [C,H,W]

    return gamma * (outH + outW) + x1


def _get_pmapped(n_dev):
    fn = _pmapped_cache.get(n_dev)
    if fn is None:
        fn = jax.pmap(
            _single,
            in_axes=(0, 0, 0, None, None, None, None, None, None, None, None, None),
            devices=jax.devices()[:n_dev],
        )
        _pmapped_cache[n_dev] = fn
    return fn


def kernel(**inputs):
    x1 = np.asarray(inputs["x1"], dtype=np.float32)
    x2 = np.asarray(inputs["x2"], dtype=np.float32)
    x3 = np.asarray(inputs["x3"], dtype=np.float32)
    Wq = np.asarray(inputs["Wq"], dtype=np.float32)
    bq = np.asarray(inputs["bq"], dtype=np.float32)
    Wk = np.asarray(inputs["Wk"], dtype=np.float32)
    bk = np.asarray(inputs["bk"], dtype=np.float32)
    Wv = np.asarray(inputs["Wv"], dtype=np.float32)
    bv = np.asarray(inputs["bv"], dtype=np.float32)
    gamma = np.float32(np.asarray(inputs["gamma"]))
    length = int(np.asarray(inputs["length"]))

    B = x1.shape[0]
    n = x1.shape[2]
    keep = (np.arange(n) < length)
    mH = (keep[:, None] & keep[None, :]).astype(np.float32)  # [H,H]
    mW = mH.copy()  # H == W

    try:
        n_dev = min(B, len(jax.devices()))
    except Exception:
        n_dev = 1
    while n_dev > 1 and B % n_dev != 0:
        n_dev -= 1
    per = B // n_dev
    if per == 1:
        fn = _get_pmapped(n_dev)
        out = fn(x1, x2, x3, Wq, bq, Wk, bk, Wv, bv, gamma, mH, mW)
    else:
        # fall back: vmap inside pmap over sub-batches
        fn = _pmapped_cache.get((n_dev, per))
        if fn is None:
            vf = jax.vmap(
                _single,
                in_axes=(0, 0, 0, None, None, None, None, None, None, None, None, None),
            )
            fn = jax.pmap(
                vf,
                in_axes=(0, 0, 0, None, None, None, None, None, None, None, None, None),
                devices=jax.devices()[:n_dev],
            )
            _pmapped_cache[(n_dev, per)] = fn
        out = fn(
            x1.reshape(n_dev, per, *x1.shape[1:]),
            x2.reshape(n_dev, per, *x2.shape[1:]),
            x3.reshape(n_dev, per, *x3.shape[1:]),
            Wq, bq, Wk, bk, Wv, bv, gamma, mH, mW,
        )
        out = np.asarray(out).reshape(B, *x1.shape[1:])
        return out.astype(np.float32)

    return np.asarray(out).astype(np.float32)
